# revision 1
# baseline (speedup 1.0000x reference)
"""Multi-head attention (B=2, S=2048, D=1024, H=16) on 8 NeuronCores.

Sharding: core c -> (batch b = c // 4, head-group g = c % 4). Each core
computes 4 heads of one batch plus the partial output projection for its
head-group's rows of Wo; the host sums the 4 partials per batch and adds bo.

Key-side compaction: masked key positions (True in `mask`) contribute
exactly zero attention weight, so the host drops them before sharding —
key/value inputs, K/V projections, score matmuls, the exp() pass and the
ctx matmuls all shrink by the masked fraction. The compacted length is
padded to a multiple of 128 with zero-columns whose mask bias (-60, applied
inside the exp activation) keeps their contribution at ~1e-26.

Layout strategy (per core):
  - Inputs are host-transposed: x^T [D, S*] so projections run with W as the
    stationary operand and x^T as the moving operand.
  - Q^T, K^T are produced in [dq, S*] layout (dq on partitions, 2 tiles of
    128 covering the 4 heads, 64 rows per head). Biases are per-partition in
    this layout and fold into the ACT evacuation (func=Identity, bias AP).
  - Scores are computed TRANSPOSED: S^T[k, q] = K Q^T, so the key-position
    (padding) mask is per-PARTITION and folds into the single exp()
    activation as a bias AP, along with the 1/sqrt(dk) scale. One exp per
    [128, 1024] PSUM tile covers both heads of a pair (the two heads' score
    matmuls run concurrently via PE row-tiling, K=64 each).
  - V is produced in natural [S*, dv] layout with a ones-column per head
    (bias folded via an augmented contraction row), so the ctx matmul
    ctx^T = [V_h | 1]^T @ P^T also yields the softmax denominator as row 64.
  - Normalization: the denominator rows are 32x32 block-transposed (bf16
    StreamTranspose) into partition-parallel form, reciprocal'd across 32
    lanes (vs 1 lane row-wise: 0.35us instead of 6.5us per unit), and
    transposed back; the row is broadcast across partitions with two K=1
    outer-product matmuls, then DVE multiplies. The whole chain is
    software-pipelined one unit behind the matmul blocks.
  - Phase interleave: DMA order xk -> xq[chunk0] -> xv -> xq[rest]; K proj
    runs first, then the attention units start as soon as q-chunk 0 is
    projected. Unit 0 runs with a deep ctx lag and carries the V projection
    in its kt slots (each v_s tile lands one slot before its ctx consumer);
    later q-chunks are projected one unit-pair ahead inside the attention
    stream. Steady-state units use ctx lag 2 so the PE never stalls on the
    exp semaphore. O-projection m-tiles are spread two per unit.

Compute dtype (env KDT): "bf16" (default) uses bfloat16 matmul operands
(~5e-3 rel err, 1 cyc/row PE + half the DMA of f32); "f32r" keeps float32r
operands (~2e-4 rel err but ~2x slower matmuls). KFP8=1 switches the score
matmuls to fp8e4m3 DoubleRow (correct, but measured slower on this hw).
"""

import os
from contextlib import ExitStack

import numpy as np

import concourse.bacc as bacc
import concourse.mybir as mybir
import concourse.tile as tile

F32 = mybir.dt.float32
F32R = mybir.dt.float32r
BF16 = mybir.dt.bfloat16
FP8 = mybir.dt.float8e4
AF = mybir.ActivationFunctionType

B, S, D = 2, 2048, 1024
H, DK = 16, 64
G = 4                    # head-groups (tensor parallel)
HPG = H // G             # 4 heads per group
DG = HPG * DK            # 256 head dims per group
NCORES = 8
MASK_NEG = -60.0         # additive post-scale bias for padded key positions
SCALE = 0.125            # 1/sqrt(dk)

KT_D = D // 128          # 8 contraction tiles for projections
NT = DG // 128           # 2 partition-tiles of qT/kT/cT (one head-pair each)
QC = 512                 # q chunk (matmul moving dim)
NQC = S // QC            # 4
VW = HPG * (DK + 1)      # 260: V width incl. per-head ones column

KDT = os.environ.get("KDT", "bf16")
# fp8e4m3 Q/K with DoubleRow-mode score matmuls (2 rows/cycle): halves the
# PE time of the largest matmul group. Softmax is insensitive to the ~0.5%
# absolute logit noise this adds (measured end-to-end rel err stays well
# under the 2e-2 gate).
# measured: DoubleRow fp8 score matmuls run at ~600ns per head vs 390ns for
# the row-tiled bf16 PAIR on this hw — a net loss, so off by default
KFP8 = os.environ.get("KFP8", "0") == "1" and KDT == "bf16"


def _dt():
    return BF16 if KDT == "bf16" else F32R


def _np_dt():
    import ml_dtypes

    return ml_dtypes.bfloat16 if KDT == "bf16" else np.float32


def build_bass(ktk):
    """Build the SPMD program for `ktk` 128-wide key tiles (SK = 128*ktk)."""
    SK = 128 * ktk
    kchunks = [(n0, min(QC, SK - n0)) for n0 in range(0, SK, QC)]
    cdt = _dt()

    nc = bacc.Bacc(None, target_bir_lowering=False, debug=False)

    xq = nc.dram_tensor("xq", [D, S], cdt, kind="ExternalInput")
    xk = nc.dram_tensor("xk", [D, SK], cdt, kind="ExternalInput")
    xv = nc.dram_tensor("xv", [D, SK], cdt, kind="ExternalInput")
    wq = nc.dram_tensor("wq", [D, DG], cdt, kind="ExternalInput")
    wk = nc.dram_tensor("wk", [D, DG], cdt, kind="ExternalInput")
    wv = nc.dram_tensor("wv", [D + 1, VW], cdt, kind="ExternalInput")
    wo = nc.dram_tensor("wo", [DG, D], cdt, kind="ExternalInput")
    bq = nc.dram_tensor("bq", [128, NT], F32, kind="ExternalInput")
    bk = nc.dram_tensor("bk", [128, NT], F32, kind="ExternalInput")
    mb = nc.dram_tensor("mb", [128, ktk], F32, kind="ExternalInput")
    cst = nc.dram_tensor("cst", [3, 128], F32R, kind="ExternalInput")
    cstc = nc.dram_tensor("cstc", [3, 128], cdt, kind="ExternalInput")
    # bf16 partials: the host sums 4 head-group partials per batch in f32,
    # so the extra rounding is ~0.1% while output DMA bytes halve
    odt = BF16 if KDT == "bf16" else F32
    out = nc.dram_tensor("out", [S, D], odt, kind="ExternalOutput")
    if KFP8:
        # scratch DRAM for the partition-pair relayout of the fp8 Q^T/K^T
        # (SBUF->SBUF DMA cannot remap partitions; DRAM-side APs can)
        q8d = nc.dram_tensor("q8d_scratch", [NT * 128, S], FP8)
        k8d = nc.dram_tensor("k8d_scratch", [NT * 128, SK], FP8)

    with tile.TileContext(nc) as tc, ExitStack() as ctx:
        consts = ctx.enter_context(tc.tile_pool(name="consts", bufs=1))
        resid = ctx.enter_context(tc.tile_pool(name="resid", bufs=1))
        stream = ctx.enter_context(tc.tile_pool(name="stream", bufs=8))
        ptp = ctx.enter_context(tc.tile_pool(name="ptp", bufs=10 if ktk <= 12 else 4))
        smalls = ctx.enter_context(tc.tile_pool(name="smalls", bufs=3 if ktk <= 12 else 2))
        obp = ctx.enter_context(tc.tile_pool(name="obp", bufs=3))

        # ---------------- constants / weights declarations ----------------
        wq_s = consts.tile([128, KT_D, DG], cdt, tag="wq_s", name="wq_s")
        bq_s = consts.tile([128, NT], F32, tag="bq_s", name="bq_s")
        wk_s = consts.tile([128, KT_D, DG], cdt, tag="wk_s", name="wk_s")
        bk_s = consts.tile([128, NT], F32, tag="bk_s", name="bk_s")
        wv_s = consts.tile([128, KT_D, VW], cdt, tag="wv_s", name="wv_s")
        wv_b = consts.tile([1, VW], cdt, tag="wv_b", name="wv_b")
        wo_s = consts.tile([128, NT, D], cdt, tag="wo_s", name="wo_s")
        mb_s = consts.tile([128, ktk], F32, tag="mb_s", name="mb_s")
        # Constant rows (all-ones, head-A selector, head-B selector) come from
        # tiny DRAM inputs — memset can't write float32r tiles.
        ones1 = consts.tile([1, 128], cdt, tag="ones1", name="ones1")
        # selector rows at partition 0, matching the reciprocal row produced
        # by the block-transpose in emit_recips; compute dtype so the
        # broadcast matmul operand dtypes match rec's
        onesA = consts.tile([1, 128], cdt, tag="onesA", name="onesA")
        onesB = consts.tile([1, 128], cdt, tag="onesB", name="onesB")

        # ---------------- input stream prefetch ----------------
        # DMA issue order sets time-to-first-exp (the ACT exp stream is the
        # kernel's long pole): xk first (K proj is the first PE work), xv
        # next (V must finish right before the first ctx matmuls), then the
        # q-chunk-0 slice of xq so the first attention unit can start while
        # the rest of xq streams in under the attention phase.
        xk_t, xv_t, xq0_t, xqr_t = [], [], [], []
        for kt in range(KT_D):
            t_ = stream.tile([128, SK], cdt, tag="xk", name="xk_s")
            nc.sync.dma_start(out=t_, in_=xk[kt * 128 : (kt + 1) * 128, :])
            xk_t.append(t_)
            if kt == 0:
                nc.sync.dma_start(
                    out=wk_s, in_=wk[:].rearrange("(kt p) n -> p kt n", p=128)
                )
                nc.sync.dma_start(out=bk_s, in_=bk[:])
                nc.sync.dma_start(out=mb_s, in_=mb[:])
        for kt in range(KT_D):
            t_ = stream.tile([128, QC], cdt, tag="xq0", name="xq0_s")
            nc.sync.dma_start(out=t_, in_=xq[kt * 128 : (kt + 1) * 128, 0:QC])
            xq0_t.append(t_)
            if kt == 0:
                nc.sync.dma_start(
                    out=wq_s, in_=wq[:].rearrange("(kt p) n -> p kt n", p=128)
                )
                nc.sync.dma_start(out=bq_s, in_=bq[:])
        for kt in range(KT_D):
            t_ = stream.tile([128, SK], cdt, tag="xv", name="xv_s")
            nc.sync.dma_start(out=t_, in_=xv[kt * 128 : (kt + 1) * 128, :])
            xv_t.append(t_)
            if kt == 0:
                nc.sync.dma_start(
                    out=wv_s, in_=wv[0:D, :].rearrange("(kt p) n -> p kt n", p=128)
                )
                nc.sync.dma_start(out=wv_b, in_=wv[D : D + 1, :])
                nc.sync.dma_start(out=ones1, in_=cstc[0:1, :])
        nc.sync.dma_start(out=onesA[0:1, :], in_=cstc[1:2, :])
        nc.sync.dma_start(out=onesB[0:1, :], in_=cstc[2:3, :])
        for kt in range(KT_D):
            t_ = stream.tile([128, S - QC], cdt, tag="xqr", name="xqr_s")
            nc.sync.dma_start(out=t_, in_=xq[kt * 128 : (kt + 1) * 128, QC:S])
            xqr_t.append(t_)
        nc.sync.dma_start(out=wo_s, in_=wo[:].rearrange("(t p) n -> p t n", p=128))

        # ---------------- resident activations ----------------
        sdt = FP8 if KFP8 else cdt
        qT = [resid.tile([128, S], sdt, tag=f"qT{t}", name=f"qT{t}") for t in range(NT)]
        kT = [resid.tile([128, SK], sdt, tag=f"kT{t}", name=f"kT{t}") for t in range(NT)]
        if KFP8:
            # partition-pair layout for DoubleRow: [q', j, c] with head-dim
            # d = 2*q' + j on 64 partitions (head A rows 0-31, B rows 32-63)
            qT8 = [
                resid.tile([64, 2, S], FP8, tag=f"q8_{t}", name=f"q8_{t}")
                for t in range(NT)
            ]
            kT8 = [
                resid.tile([64, 2, SK], FP8, tag=f"k8_{t}", name=f"k8_{t}")
                for t in range(NT)
            ]
        v_s = resid.tile([128, ktk, VW], cdt, tag="v_s", name="v_s")
        cT = [resid.tile([128, S], cdt, tag=f"cT{t}", name=f"cT{t}") for t in range(NT)]

        # ---------------- phase 1: K^T projection ----------------
        with tc.tile_pool(name="pp", bufs=1, space="PSUM") as pp:
            psums = [
                pp.tile([128, QC], F32, tag=f"pp{i}", name=f"pp{i}")
                for i in range(NT * len(kchunks))
            ]
            for kt in range(KT_D):
                for t in range(NT):
                    for ci, (n0, w) in enumerate(kchunks):
                        nc.tensor.matmul(
                            psums[t * len(kchunks) + ci][:, 0:w],
                            lhsT=wk_s[:, kt, t * 128 : (t + 1) * 128],
                            rhs=xk_t[kt][:, n0 : n0 + w],
                            start=(kt == 0),
                            stop=(kt == KT_D - 1),
                        )
            for t in range(NT):
                for ci, (n0, w) in enumerate(kchunks):
                    nc.scalar.activation(
                        out=kT[t][:, n0 : n0 + w],
                        in_=psums[t * len(kchunks) + ci][:, 0:w],
                        func=AF.Identity,
                        bias=bk_s[:, t : t + 1],
                        scale=1.0,
                    )
            if KFP8:
                for t in range(NT):
                    rows = slice(t * 128, (t + 1) * 128)
                    nc.sync.dma_start(out=k8d[rows, :], in_=kT[t][:])
                    nc.sync.dma_start(
                        out=kT8[t][:],
                        in_=k8d[rows, :].rearrange("(q j) c -> q j c", j=2),
                    )

        # ------- phases 2-4: V + Q projections woven into attention -------
        with tc.tile_pool(name="pa", bufs=1, space="PSUM") as pa:
            units = [(qc, p) for qc in range(NQC) for p in range(NT)]
            pend = {}

            def emit_vproj(m):
                # one V m-tile (all 4 heads + ones column); hooked into the
                # first unit's kt loop so it rides under the exp stream
                pvm = pa.tile([128, VW], F32, tag="ps", bufs=2, name="pv")
                for kt in range(KT_D):
                    nc.tensor.matmul(
                        pvm[:],
                        lhsT=xv_t[kt][:, m * 128 : (m + 1) * 128],
                        rhs=wv_s[:, kt, :],
                        start=(kt == 0),
                        stop=False,
                    )
                # bias + ones columns via augmented K=1 row
                nc.tensor.matmul(
                    pvm[:], lhsT=ones1[:], rhs=wv_b[:], start=False, stop=True
                )
                nc.vector.tensor_copy(v_s[:, m, :], pvm[:])

            def emit_qproj(qc):
                # one q-chunk of the Q projection, one head pair at a time:
                # two short psum-slot holds (~1us each) instead of one long
                # one, so interleaved score matmuls can slip between them
                qsl = slice(qc * QC, (qc + 1) * QC)
                for t in range(NT):
                    qp = pa.tile([128, QC], F32, tag="ps", bufs=2, name="qp")
                    for kt in range(KT_D):
                        xsrc = (
                            xq0_t[kt][:, 0:QC]
                            if qc == 0
                            else xqr_t[kt][:, (qc - 1) * QC : qc * QC]
                        )
                        nc.tensor.matmul(
                            qp[:],
                            lhsT=wq_s[:, kt, t * 128 : (t + 1) * 128],
                            rhs=xsrc,
                            start=(kt == 0),
                            stop=(kt == KT_D - 1),
                        )
                    # DVE evac (bias add + cast): ACT's exp backlog would
                    # hold the qp psum slot hostage for several us
                    nc.vector.tensor_scalar_add(
                        qT[t][:, qsl], qp[:], bq_s[:, t : t + 1]
                    )
                if KFP8:
                    for t in range(NT):
                        rows = slice(t * 128, (t + 1) * 128)
                        nc.sync.dma_start(
                            out=q8d[rows, qsl], in_=qT[t][:, qsl]
                        )
                        nc.sync.dma_start(
                            out=qT8[t][:, :, qsl],
                            in_=q8d[rows, qsl].rearrange(
                                "(q j) c -> q j c", j=2
                            ),
                        )

            def emit_attn(qc, p, hook=None, lag=1):
                qsl = slice(qc * QC, (qc + 1) * QC)
                hA, hB = 2 * p, 2 * p + 1
                pcA = pa.tile([65, QC], F32, tag="pcA", bufs=2, name="pcA")
                pcB = pa.tile([65, QC], F32, tag="pcB", bufs=2, name="pcB")
                pts = []
                # software pipeline: scores/exp `lag` kts ahead of ctx
                LAG = lag
                for kt in range(ktk + LAG):
                    if kt < ktk:
                        ksl = slice(kt * 128, (kt + 1) * 128)
                        ps = pa.tile([128, 2 * QC], F32, tag="ps", bufs=2, name="ps")
                        if KFP8:
                            MPD = mybir.MatmulPerfMode.DoubleRow
                            nc.tensor.matmul(
                                ps[:, 0:QC],
                                lhsT=kT8[p][0:32, :, ksl],
                                rhs=qT8[p][0:32, :, qsl],
                                start=True,
                                stop=True,
                                perf_mode=MPD,
                            )
                            nc.tensor.matmul(
                                ps[:, QC : 2 * QC],
                                lhsT=kT8[p][32:64, :, ksl],
                                rhs=qT8[p][32:64, :, qsl],
                                start=True,
                                stop=True,
                                perf_mode=MPD,
                            )
                        else:
                            nc.tensor.matmul(
                                ps[:, 0:QC],
                                lhsT=kT[p][0:64, ksl],
                                rhs=qT[p][0:64, qsl],
                                start=True,
                                stop=True,
                            )
                            nc.tensor.matmul(
                                ps[:, QC : 2 * QC],
                                lhsT=kT[p][64:128, ksl],
                                rhs=qT[p][64:128, qsl],
                                start=True,
                                stop=True,
                            )
                        pt = ptp.tile([128, 2 * QC], cdt, tag="pt", name="pt")
                        nc.scalar.activation(
                            out=pt[:],
                            in_=ps[:],
                            func=AF.Exp,
                            bias=mb_s[:, kt : kt + 1],
                            scale=SCALE,
                        )
                        pts.append(pt)
                    if hook is not None and kt < ktk + LAG - 1:
                        hook(kt)
                    if kt >= LAG:
                        ct = kt - LAG
                        ptc = pts[ct]
                        nc.tensor.matmul(
                            pcA[0:65, :],
                            lhsT=v_s[:, ct, hA * 65 : (hA + 1) * 65],
                            rhs=ptc[:, 0:QC],
                            start=(ct == 0),
                            stop=(ct == ktk - 1),
                        )
                        nc.tensor.matmul(
                            pcB[0:65, :],
                            lhsT=v_s[:, ct, hB * 65 : (hB + 1) * 65],
                            rhs=ptc[:, QC : 2 * QC],
                            start=(ct == 0),
                            stop=(ct == ktk - 1),
                        )
                return pcA, pcB

            def emit_recips(pcA, pcB):
                if cdt != BF16:
                    rec = smalls.tile([1, 2 * QC], F32R, tag="rec", name="rec")
                    with nc.allow_low_precision(reason="fp32r PE broadcast"):
                        nc.vector.reciprocal(rec[0:1, 0:QC], pcA[64:65, :])
                        nc.vector.reciprocal(
                            rec[0:1, QC : 2 * QC], pcB[64:65, :]
                        )
                    return rec
                # InstReciprocal costs ~6.4ns/elem on a single partition, so
                # reciprocal of [1,1024] rows costs 6.5us/unit on the DVE.
                # Instead: 32x32 block-transpose the denominator row into
                # partition-parallel form (dens land in the stride-32
                # columns), reciprocal there across 32 lanes, and transpose
                # back to a row for the PE broadcast. StreamTranspose is
                # SBUF-only and 2-byte-only, hence the bf16 staging (~0.4%
                # on the normalizer, well inside the error budget).
                std = smalls.tile([32, 2 * QC], BF16, tag="std", name="std")
                stg = smalls.tile([32, 2 * QC], BF16, tag="stg", name="stg")
                sre = smalls.tile([32, 2 * QC], BF16, tag="sre", name="sre")
                rec = smalls.tile([32, 2 * QC], BF16, tag="rec", name="rec")
                nc.vector.tensor_copy(std[0:1, 0:QC], pcA[64:65, :])
                nc.vector.tensor_copy(std[0:1, QC : 2 * QC], pcB[64:65, :])
                nc.vector.transpose(stg[:], std[:])
                stg_d = stg[:].rearrange("p (b c) -> p b c", c=32)[:, :, 0:1]
                sre_d = sre[:].rearrange("p (b c) -> p b c", c=32)[:, :, 0:1]
                with nc.allow_low_precision(reason="bf16 softmax normalizer"):
                    nc.vector.reciprocal(sre_d, stg_d)
                nc.vector.transpose(rec[:], sre[:])
                return rec

            def emit_norm(qc, p, pcA, pcB, rec):
                qsl = slice(qc * QC, (qc + 1) * QC)
                pbc = pa.tile([128, QC], F32, tag="ps", bufs=2, name="pbc")
                nc.tensor.matmul(
                    pbc[:],
                    lhsT=onesA[0:1, :],
                    rhs=rec[0:1, 0:QC],
                    start=True,
                    stop=False,
                )
                nc.tensor.matmul(
                    pbc[:],
                    lhsT=onesB[0:1, :],
                    rhs=rec[0:1, QC : 2 * QC],
                    start=False,
                    stop=True,
                )
                bcs = smalls.tile([128, QC], F32, tag="bcs", name="bcs")
                nc.vector.tensor_copy(bcs[:], pbc[:])
                nc.vector.tensor_mul(cT[p][0:64, qsl], pcA[0:64, :], bcs[0:64, :])
                nc.vector.tensor_mul(
                    cT[p][64:128, qsl], pcB[0:64, :], bcs[64:128, :]
                )

            def emit_final(ms):
                for m in ms:
                    pom = pa.tile([128, D], F32, tag="ps", bufs=2, name="pom")
                    for oc in range(2):
                        for t in range(NT):
                            nc.tensor.matmul(
                                pom[:, oc * 512 : (oc + 1) * 512],
                                lhsT=cT[t][:, m * 128 : (m + 1) * 128],
                                rhs=wo_s[:, t, oc * 512 : (oc + 1) * 512],
                                start=(t == 0),
                                stop=(t == NT - 1),
                            )
                    ob = obp.tile([128, D], odt, tag="ob", name="ob")
                    # DVE copy (gpsimd can't read PSUM): ACT is saturated by
                    # the exp() stream; DVE has slack once the reciprocals
                    # use the fast approximation
                    nc.vector.tensor_copy(ob[:], pom[:])
                    nc.sync.dma_start(out=out[m * 128 : (m + 1) * 128, :], in_=ob[:])

            # Unit 0 runs with a deep ctx lag: its exp stream starts as soon
            # as xq chunk 0 + kT are ready (~8us before xv finishes), and
            # the V projection tiles are hooked in just after xv lands, each
            # one kt slot ahead of the ctx matmul that consumes it.
            LAG0 = min(4, ktk)

            def vhook(kt):
                if LAG0 - 1 <= kt < LAG0 - 1 + ktk:
                    emit_vproj(kt - (LAG0 - 1))

            emit_qproj(0)
            for i, (qc, p) in enumerate(units):
                hook = vhook if i == 0 else None
                pcA, pcB = emit_attn(qc, p, hook, lag=LAG0 if i == 0 else 2)
                if i >= 3:
                    # two O-proj m-tiles per unit, emitted FIRST so their
                    # psum slots evacuate before the DVE gets busy with the
                    # norm chain (the next unit's scores wait on these slots)
                    emit_final([2 * (i - 3), 2 * (i - 3) + 1])
                if i >= 1:
                    pqc, pp_ = units[i - 1]
                    emit_norm(pqc, pp_, *pend[i - 1])
                if i % 2 == 1 and qc < NQC - 1:
                    # project the next q-chunk one unit-pair ahead; the PE
                    # insert hides under the trailing exp queue
                    emit_qproj(qc + 1)
                # recips for this unit go last so the DVE queue serves the
                # previous unit's normalization and evacuations first
                pend[i] = (pcA, pcB, emit_recips(pcA, pcB))
            lqc, lp = units[-1]
            emit_norm(lqc, lp, *pend[len(units) - 1])
            emit_final(range(10, 16))

    nc.compile()
    return nc


def _const_rows():
    cst = np.zeros((3, 128), np.float32)
    cst[0, :] = 1.0
    cst[1, 0:64] = 1.0
    cst[2, 64:128] = 1.0
    return cst


def make_in_maps(query, key, value, mask, Wq, bq, Wk, bk, Wv, bv, Wo, bo):
    """Returns (in_maps, ktk). Key positions with mask=True are dropped."""
    query = np.asarray(query, np.float32)
    key = np.asarray(key, np.float32)
    value = np.asarray(value, np.float32)
    mask = np.asarray(mask)
    Wq = np.asarray(Wq, np.float32)
    Wk = np.asarray(Wk, np.float32)
    Wv = np.asarray(Wv, np.float32)
    Wo = np.asarray(Wo, np.float32)
    bq = np.asarray(bq, np.float32)
    bk = np.asarray(bk, np.float32)
    bv = np.asarray(bv, np.float32)

    keep = [np.flatnonzero(~mask[b, 0]) for b in range(B)]
    ktk = max(1, max((len(k) + 127) // 128 for k in keep))
    SKc = 128 * ktk
    ndt = _np_dt()

    in_maps = []
    for c in range(NCORES):
        b, g = c // G, c % G
        cs = slice(g * DG, (g + 1) * DG)
        idx = keep[b]
        nk = len(idx)
        xkc = np.zeros((D, SKc), np.float32)
        xvc = np.zeros((D, SKc), np.float32)
        xkc[:, :nk] = key[b].T[:, idx]
        xvc[:, :nk] = value[b].T[:, idx]
        mbias = np.full(SKc, MASK_NEG, np.float32)
        mbias[:nk] = 0.0

        wv_aug = np.zeros((D + 1, VW), np.float32)
        for j in range(HPG):
            src = slice(g * DG + j * DK, g * DG + (j + 1) * DK)
            wv_aug[:D, j * 65 : j * 65 + 64] = Wv[:, src]
            wv_aug[D, j * 65 : j * 65 + 64] = bv[src]
            wv_aug[D, j * 65 + 64] = 1.0

        in_maps.append(
            {
                "xq": np.ascontiguousarray(query[b].T).astype(ndt),
                "xk": xkc.astype(ndt),
                "xv": xvc.astype(ndt),
                "wq": np.ascontiguousarray(Wq[:, cs]).astype(ndt),
                "wk": np.ascontiguousarray(Wk[:, cs]).astype(ndt),
                "wv": wv_aug.astype(ndt),
                "wo": np.ascontiguousarray(Wo[cs, :]).astype(ndt),
                "bq": np.ascontiguousarray(bq[cs].reshape(NT, 128).T),
                "bk": np.ascontiguousarray(bk[cs].reshape(NT, 128).T),
                "mb": np.ascontiguousarray(mbias.reshape(ktk, 128).T),
                "cst": _const_rows(),
                "cstc": _const_rows().astype(ndt),
            }
        )
    return in_maps, ktk


def combine_outputs(results, mask, bo):
    mask = np.asarray(mask)
    bo = np.asarray(bo, np.float32)
    out = np.zeros((B, S, D), np.float32)
    for c in range(NCORES):
        out[c // G] += np.asarray(results[c]["out"], np.float32)
    for b in range(B):
        if mask[b, 0].all():
            # reference: fully-masked rows produce zero context
            out[b] = 0.0
    out += bo[None, None, :]
    return out


_NC_CACHE = {}


def kernel(query, key, value, mask, Wq, bq, Wk, bk, Wv, bv, Wo, bo):
    from concourse.bass_utils import run_bass_kernel_spmd

    in_maps, ktk = make_in_maps(
        query, key, value, mask, Wq, bq, Wk, bk, Wv, bv, Wo, bo
    )
    nc = _NC_CACHE.get((KDT, ktk))
    if nc is None:
        nc = _NC_CACHE[(KDT, ktk)] = build_bass(ktk)
    res = run_bass_kernel_spmd(nc, in_maps, list(range(NCORES))).results
    return combine_outputs(res, mask, bo)



# revision 10
# speedup vs baseline: 1.1381x; 1.1381x over previous
"""Multi-head attention (B=2, S=2048, D=1024, H=16) on 8 NeuronCores.

Sharding: core c -> (batch b = c // 4, head-group g = c % 4). Each core
computes 4 heads of one batch plus the partial output projection for its
head-group's rows of Wo; the host sums the 4 partials per batch and adds bo.

Key-side compaction: masked key positions (True in `mask`) contribute
exactly zero attention weight, so the host drops them before sharding —
key/value inputs, K/V projections, score matmuls, the exp() pass and the
ctx matmuls all shrink by the masked fraction. The compacted length is
padded to a multiple of 128 with zero-columns whose mask bias (-60, applied
inside the exp activation) keeps their contribution at ~1e-26.

Layout strategy (per core):
  - Inputs are host-transposed: x^T [D, S*] so projections run with W as the
    stationary operand and x^T as the moving operand. Weights are
    host-prearranged into the [128, kt, n] SBUF layout so their DMA is one
    contiguous descriptor per partition row.
  - Q^T, K^T are produced in [dq, S*] layout (dq on partitions, 2 tiles of
    128 covering the 4 heads, 64 rows per head). Biases are per-partition in
    this layout and fold into the ACT evacuation (func=Identity, bias AP).
  - Scores are computed TRANSPOSED: S^T[k, q] = K Q^T, so the key-position
    (padding) mask is per-PARTITION and folds into the single exp()
    activation as a bias AP, along with the 1/sqrt(dk) scale. One exp per
    [128, 2*w] PSUM tile covers both heads of a pair (the two heads' score
    matmuls run concurrently via PE row-tiling, K=64 each).
  - V is produced in natural [S*, dv] layout with a ones-column per head
    (bias folded via an augmented contraction row), so the ctx matmul
    ctx^T = [V_h | 1]^T @ P^T also yields the softmax denominator as row 64.
  - Normalization: the denominator rows are reciprocal'd in place with the
    custom-DVE fast approximation (~18-bit, one instruction per head —
    replaces the previous 32x32 block-transpose + InstReciprocal chain),
    cast to the compute dtype, broadcast across partitions with two K=1
    outer-product matmuls, then DVE multiplies. The whole chain is
    software-pipelined one unit behind the matmul blocks.
  - Phase interleave: DMA order wk -> xk -> wq -> xq[chunk0] -> wv -> xv ->
    xq[rest] -> wo; K proj runs first, then the attention units start as
    soon as q-chunk 0 is projected. Unit 0 runs with a deep ctx lag and
    carries the V projection in its kt slots (each v_s tile lands one slot
    before its ctx consumer); later q-chunks are projected one unit-pair
    ahead inside the attention stream. Steady-state units use ctx lag 2 so
    the PE never stalls on the exp semaphore.
  - Tail: the last 512-wide q-chunk is split into two 256-wide units and
    O-projection m-tiles are spread two per unit on an eligibility-driven
    schedule, so only the final two m-tiles (256 q rows) remain after the
    last unit's normalization instead of six.

Compute dtype (env KDT): "bf16" (default) uses bfloat16 matmul operands
(~5e-3 rel err, 1 cyc/row PE + half the DMA of f32); "f32r" keeps float32r
operands (~2e-4 rel err but ~2x slower matmuls).
"""

import os
from contextlib import ExitStack

import numpy as np

import concourse.bacc as bacc
import concourse.mybir as mybir
import concourse.tile as tile

F32 = mybir.dt.float32
F32R = mybir.dt.float32r
BF16 = mybir.dt.bfloat16
AF = mybir.ActivationFunctionType

B, S, D = 2, 2048, 1024
H, DK = 16, 64
G = 4                    # head-groups (tensor parallel)
HPG = H // G             # 4 heads per group
DG = HPG * DK            # 256 head dims per group
NCORES = 8
MASK_NEG = -60.0         # additive post-scale bias for padded key positions
SCALE = 0.125            # 1/sqrt(dk)

KT_D = D // 128          # 8 contraction tiles for projections
NT = DG // 128           # 2 partition-tiles of qT/kT/cT (one head-pair each)
QC = 512                 # q projection chunk (matmul moving dim)
NQC = S // QC            # 4
# attention-unit q chunks: last 512 chunk split in two so the tail after the
# final normalization is only two O-proj m-tiles
if os.environ.get("KTAIL", "1") == "1":
    QCHUNKS = [(0, 512), (512, 512), (1024, 512), (1536, 256), (1792, 256)]
else:
    QCHUNKS = [(0, 512), (512, 512), (1024, 512), (1536, 512)]
VW = HPG * (DK + 1)      # 260: V width incl. per-head ones column

KDT = os.environ.get("KDT", "bf16")


def _dt():
    return BF16 if KDT == "bf16" else F32R


def _np_dt():
    import ml_dtypes

    return ml_dtypes.bfloat16 if KDT == "bf16" else np.float32


def build_bass(ktk):
    """Build the SPMD program for `ktk` 128-wide key tiles (SK = 128*ktk)."""
    SK = 128 * ktk
    kchunks = [(n0, min(QC, SK - n0)) for n0 in range(0, SK, QC)]
    cdt = _dt()

    nc = bacc.Bacc(None, target_bir_lowering=False, debug=False)

    xq = nc.dram_tensor("xq", [D, S], cdt, kind="ExternalInput")
    xk = nc.dram_tensor("xk", [D, SK], cdt, kind="ExternalInput")
    xv = nc.dram_tensor("xv", [D, SK], cdt, kind="ExternalInput")
    # weights pre-arranged host-side into the SBUF tile layout (contiguous
    # per-partition rows -> few large DMA descriptors)
    wq = nc.dram_tensor("wq", [128, KT_D, DG], cdt, kind="ExternalInput")
    wk = nc.dram_tensor("wk", [128, KT_D, DG], cdt, kind="ExternalInput")
    wv = nc.dram_tensor("wv", [128, KT_D, VW], cdt, kind="ExternalInput")
    wvb = nc.dram_tensor("wvb", [1, VW], cdt, kind="ExternalInput")
    wo = nc.dram_tensor("wo", [128, NT, D], cdt, kind="ExternalInput")
    bq = nc.dram_tensor("bq", [128, NT], F32, kind="ExternalInput")
    bk = nc.dram_tensor("bk", [128, NT], F32, kind="ExternalInput")
    mb = nc.dram_tensor("mb", [128, ktk], F32, kind="ExternalInput")
    cstc = nc.dram_tensor("cstc", [3, 128], cdt, kind="ExternalInput")
    # bf16 partials: the host sums 4 head-group partials per batch in f32,
    # so the extra rounding is ~0.1% while output DMA bytes halve
    odt = BF16 if KDT == "bf16" else F32
    out = nc.dram_tensor("out", [S, D], odt, kind="ExternalOutput")

    with tile.TileContext(nc) as tc, ExitStack() as ctx:
        consts = ctx.enter_context(tc.tile_pool(name="consts", bufs=1))
        resid = ctx.enter_context(tc.tile_pool(name="resid", bufs=1))
        stream = ctx.enter_context(tc.tile_pool(name="stream", bufs=8))
        ptp = ctx.enter_context(tc.tile_pool(name="ptp", bufs=10 if ktk <= 12 else 4))
        smalls = ctx.enter_context(tc.tile_pool(name="smalls", bufs=3 if ktk <= 12 else 2))
        obp = ctx.enter_context(tc.tile_pool(name="obp", bufs=3))

        # ---------------- constants / weights declarations ----------------
        wq_s = consts.tile([128, KT_D, DG], cdt, tag="wq_s", name="wq_s")
        bq_s = consts.tile([128, NT], F32, tag="bq_s", name="bq_s")
        wk_s = consts.tile([128, KT_D, DG], cdt, tag="wk_s", name="wk_s")
        bk_s = consts.tile([128, NT], F32, tag="bk_s", name="bk_s")
        wv_s = consts.tile([128, KT_D, VW], cdt, tag="wv_s", name="wv_s")
        wv_b = consts.tile([1, VW], cdt, tag="wv_b", name="wv_b")
        wo_s = consts.tile([128, NT, D], cdt, tag="wo_s", name="wo_s")
        mb_s = consts.tile([128, ktk], F32, tag="mb_s", name="mb_s")
        # Constant rows (all-ones, head-A selector, head-B selector) come from
        # tiny DRAM inputs — memset can't write float32r tiles.
        ones1 = consts.tile([1, 128], cdt, tag="ones1", name="ones1")
        onesA = consts.tile([1, 128], cdt, tag="onesA", name="onesA")
        onesB = consts.tile([1, 128], cdt, tag="onesB", name="onesB")

        # ---------------- input stream prefetch ----------------
        # DMA issue order sets time-to-first-exp (the ACT exp stream is the
        # kernel's long pole): wk+xk first (K proj is the first PE work),
        # then wq + the q-chunk-0 slice of xq (first attention unit), xv
        # (V must finish right before the first ctx matmuls), then the rest
        # of xq streaming in under the attention phase.
        nc.sync.dma_start(out=wk_s, in_=wk[:])
        nc.sync.dma_start(out=bk_s, in_=bk[:])
        nc.sync.dma_start(out=mb_s, in_=mb[:])
        xk_t, xv_t, xq0_t, xqr_t = [], [], [], []
        for kt in range(KT_D):
            t_ = stream.tile([128, SK], cdt, tag="xk", name="xk_s")
            nc.sync.dma_start(out=t_, in_=xk[kt * 128 : (kt + 1) * 128, :])
            xk_t.append(t_)
        nc.sync.dma_start(out=wq_s, in_=wq[:])
        nc.sync.dma_start(out=bq_s, in_=bq[:])
        for kt in range(KT_D):
            t_ = stream.tile([128, QC], cdt, tag="xq0", name="xq0_s")
            nc.sync.dma_start(out=t_, in_=xq[kt * 128 : (kt + 1) * 128, 0:QC])
            xq0_t.append(t_)
        nc.sync.dma_start(out=wv_s, in_=wv[:])
        nc.sync.dma_start(out=wv_b, in_=wvb[:])
        nc.sync.dma_start(out=ones1, in_=cstc[0:1, :])
        for kt in range(KT_D):
            t_ = stream.tile([128, SK], cdt, tag="xv", name="xv_s")
            nc.sync.dma_start(out=t_, in_=xv[kt * 128 : (kt + 1) * 128, :])
            xv_t.append(t_)
        nc.sync.dma_start(out=onesA[0:1, :], in_=cstc[1:2, :])
        nc.sync.dma_start(out=onesB[0:1, :], in_=cstc[2:3, :])
        for kt in range(KT_D):
            t_ = stream.tile([128, S - QC], cdt, tag="xqr", name="xqr_s")
            nc.sync.dma_start(out=t_, in_=xq[kt * 128 : (kt + 1) * 128, QC:S])
            xqr_t.append(t_)
        nc.sync.dma_start(out=wo_s, in_=wo[:])

        # ---------------- resident activations ----------------
        qT = [resid.tile([128, S], cdt, tag=f"qT{t}", name=f"qT{t}") for t in range(NT)]
        kT = [resid.tile([128, SK], cdt, tag=f"kT{t}", name=f"kT{t}") for t in range(NT)]
        v_s = resid.tile([128, ktk, VW], cdt, tag="v_s", name="v_s")
        cT = [resid.tile([128, S], cdt, tag=f"cT{t}", name=f"cT{t}") for t in range(NT)]

        # ---------------- phase 1: K^T projection ----------------
        with tc.tile_pool(name="pp", bufs=1, space="PSUM") as pp:
            psums = [
                pp.tile([128, QC], F32, tag=f"pp{i}", name=f"pp{i}")
                for i in range(NT * len(kchunks))
            ]
            for kt in range(KT_D):
                for t in range(NT):
                    for ci, (n0, w) in enumerate(kchunks):
                        nc.tensor.matmul(
                            psums[t * len(kchunks) + ci][:, 0:w],
                            lhsT=wk_s[:, kt, t * 128 : (t + 1) * 128],
                            rhs=xk_t[kt][:, n0 : n0 + w],
                            start=(kt == 0),
                            stop=(kt == KT_D - 1),
                        )
            for t in range(NT):
                for ci, (n0, w) in enumerate(kchunks):
                    nc.scalar.activation(
                        out=kT[t][:, n0 : n0 + w],
                        in_=psums[t * len(kchunks) + ci][:, 0:w],
                        func=AF.Identity,
                        bias=bk_s[:, t : t + 1],
                        scale=1.0,
                    )

        # ------- phases 2-4: V + Q projections woven into attention -------
        with tc.tile_pool(name="pa", bufs=1, space="PSUM") as pa:
            units = [(q0, w, p) for (q0, w) in QCHUNKS for p in range(NT)]
            pend = {}

            def emit_vproj(m):
                # one V m-tile (all 4 heads + ones column); hooked into the
                # first unit's kt loop so it rides under the exp stream
                pvm = pa.tile([128, VW], F32, tag="ps", bufs=2, name="pv")
                for kt in range(KT_D):
                    nc.tensor.matmul(
                        pvm[:],
                        lhsT=xv_t[kt][:, m * 128 : (m + 1) * 128],
                        rhs=wv_s[:, kt, :],
                        start=(kt == 0),
                        stop=False,
                    )
                # bias + ones columns via augmented K=1 row
                nc.tensor.matmul(
                    pvm[:], lhsT=ones1[:], rhs=wv_b[:], start=False, stop=True
                )
                nc.vector.tensor_copy(v_s[:, m, :], pvm[:])

            def emit_qproj(qc):
                # one q-chunk of the Q projection, one head pair at a time:
                # two short psum-slot holds (~1us each) instead of one long
                # one, so interleaved score matmuls can slip between them
                qsl = slice(qc * QC, (qc + 1) * QC)
                for t in range(NT):
                    qp = pa.tile([128, QC], F32, tag="ps", bufs=2, name="qp")
                    for kt in range(KT_D):
                        xsrc = (
                            xq0_t[kt][:, 0:QC]
                            if qc == 0
                            else xqr_t[kt][:, (qc - 1) * QC : qc * QC]
                        )
                        nc.tensor.matmul(
                            qp[:],
                            lhsT=wq_s[:, kt, t * 128 : (t + 1) * 128],
                            rhs=xsrc,
                            start=(kt == 0),
                            stop=(kt == KT_D - 1),
                        )
                    # DVE evac (bias add + cast): ACT's exp backlog would
                    # hold the qp psum slot hostage for several us
                    nc.vector.tensor_scalar_add(
                        qT[t][:, qsl], qp[:], bq_s[:, t : t + 1]
                    )

            def emit_attn(q0, w, p, hook=None, lag=1):
                qsl = slice(q0, q0 + w)
                hA, hB = 2 * p, 2 * p + 1
                pcA = pa.tile([65, w], F32, tag="pcA", bufs=2, name="pcA")
                pcB = pa.tile([65, w], F32, tag="pcB", bufs=2, name="pcB")
                pts = []
                # software pipeline: scores/exp `lag` kts ahead of ctx
                LAG = lag
                for kt in range(ktk + LAG):
                    if kt < ktk:
                        ksl = slice(kt * 128, (kt + 1) * 128)
                        # per-head row stride padded to QC so both heads'
                        # matmul destinations stay PSUM-bank-aligned even
                        # for the 256-wide tail units
                        ps = pa.tile([128, 2, QC], F32, tag="ps", bufs=2, name="ps")
                        nc.tensor.matmul(
                            ps[:, 0, 0:w],
                            lhsT=kT[p][0:64, ksl],
                            rhs=qT[p][0:64, qsl],
                            start=True,
                            stop=True,
                        )
                        nc.tensor.matmul(
                            ps[:, 1, 0:w],
                            lhsT=kT[p][64:128, ksl],
                            rhs=qT[p][64:128, qsl],
                            start=True,
                            stop=True,
                        )
                        pt = ptp.tile([128, 2, w], cdt, tag="pt", name="pt")
                        nc.scalar.activation(
                            out=pt[:],
                            in_=ps[:, :, 0:w],
                            func=AF.Exp,
                            bias=mb_s[:, kt : kt + 1],
                            scale=SCALE,
                        )
                        pts.append(pt)
                    if hook is not None and kt < ktk + LAG - 1:
                        hook(kt)
                    if kt >= LAG:
                        ct = kt - LAG
                        ptc = pts[ct]
                        nc.tensor.matmul(
                            pcA[0:65, :],
                            lhsT=v_s[:, ct, hA * 65 : (hA + 1) * 65],
                            rhs=ptc[:, 0, :],
                            start=(ct == 0),
                            stop=(ct == ktk - 1),
                        )
                        nc.tensor.matmul(
                            pcB[0:65, :],
                            lhsT=v_s[:, ct, hB * 65 : (hB + 1) * 65],
                            rhs=ptc[:, 1, :],
                            start=(ct == 0),
                            stop=(ct == ktk - 1),
                        )
                return pcA, pcB

            def emit_recips(pcA, pcB, w):
                if os.environ.get("KREC", "fast") == "old":
                    # baseline path: 32x32 block-transpose + InstReciprocal
                    std = smalls.tile([32, 2 * QC], BF16, tag="std", name="std")
                    stg = smalls.tile([32, 2 * QC], BF16, tag="stg", name="stg")
                    sre = smalls.tile([32, 2 * QC], BF16, tag="sre", name="sre")
                    rec = smalls.tile([32, 2 * QC], BF16, tag="rec", name="rec")
                    nc.vector.tensor_copy(std[0:1, 0:w], pcA[64:65, :])
                    nc.vector.tensor_copy(std[0:1, w : 2 * w], pcB[64:65, :])
                    nc.vector.transpose(stg[:, 0 : 2 * w], std[:, 0 : 2 * w])
                    stg_d = stg[:, 0 : 2 * w].rearrange("p (b c) -> p b c", c=32)[:, :, 0:1]
                    sre_d = sre[:, 0 : 2 * w].rearrange("p (b c) -> p b c", c=32)[:, :, 0:1]
                    with nc.allow_low_precision(reason="bf16 softmax normalizer"):
                        nc.vector.reciprocal(sre_d, stg_d)
                    nc.vector.transpose(rec[:, 0 : 2 * w], sre[:, 0 : 2 * w])
                    return rec
                # fast-approx reciprocal (custom DVE, ~18 bits) on the
                # denominator rows (copied to SBUF), then one cast to the
                # compute dtype for the PE broadcast
                den = smalls.tile([1, 2 * QC], F32, tag="den", name="den")
                rec32 = smalls.tile([1, 2 * QC], F32, tag="rec32", name="rec32")
                rec = smalls.tile([1, 2 * QC], cdt, tag="rec", name="rec")
                nc.vector.tensor_copy(den[0:1, 0:w], pcA[64:65, 0:w])
                nc.vector.tensor_copy(den[0:1, w : 2 * w], pcB[64:65, 0:w])
                nc.vector.reciprocal_approx_fast(
                    out=rec32[0:1, 0 : 2 * w], in_=den[0:1, 0 : 2 * w]
                )
                nc.vector.tensor_copy(rec[0:1, 0 : 2 * w], rec32[0:1, 0 : 2 * w])
                return rec

            def emit_norm(q0, w, p, pcA, pcB, rec):
                qsl = slice(q0, q0 + w)
                pbc = pa.tile([128, w], F32, tag="ps", bufs=2, name="pbc")
                nc.tensor.matmul(
                    pbc[:],
                    lhsT=onesA[0:1, :],
                    rhs=rec[0:1, 0:w],
                    start=True,
                    stop=False,
                )
                nc.tensor.matmul(
                    pbc[:],
                    lhsT=onesB[0:1, :],
                    rhs=rec[0:1, w : 2 * w],
                    start=False,
                    stop=True,
                )
                bcs = smalls.tile([128, QC], F32, tag="bcs", name="bcs")
                nc.vector.tensor_copy(bcs[:, 0:w], pbc[:])
                nc.vector.tensor_mul(cT[p][0:64, qsl], pcA[0:64, :], bcs[0:64, 0:w])
                nc.vector.tensor_mul(
                    cT[p][64:128, qsl], pcB[0:64, :], bcs[64:128, 0:w]
                )

            def emit_final(ms):
                for m in ms:
                    pom = pa.tile([128, D], F32, tag="ps", bufs=2, name="pom")
                    # t-outer so each cT stationary is loaded once for both
                    # 512-wide output column halves
                    if os.environ.get("KFIN", "1") == "1":
                        loop = [(t, oc) for t in range(NT) for oc in range(2)]
                    else:
                        loop = [(t, oc) for oc in range(2) for t in range(NT)]
                    for t, oc in loop:
                        nc.tensor.matmul(
                            pom[:, oc * 512 : (oc + 1) * 512],
                            lhsT=cT[t][:, m * 128 : (m + 1) * 128],
                            rhs=wo_s[:, t, oc * 512 : (oc + 1) * 512],
                            start=(t == 0),
                            stop=(t == NT - 1),
                        )
                    ob = obp.tile([128, D], odt, tag="ob", name="ob")
                    # DVE copy (gpsimd can't read PSUM): ACT is saturated by
                    # the exp() stream
                    nc.vector.tensor_copy(ob[:], pom[:])
                    nc.sync.dma_start(out=out[m * 128 : (m + 1) * 128, :], in_=ob[:])

            # O-projection m-tile schedule: eligibility-driven spread, two
            # per unit; m-tiles of q-chunk c are ready at iteration 2c+3
            # (both its units norm'd), leaving only m14,15 after the loop.
            if len(QCHUNKS) == 5 and os.environ.get("KOS", "1") == "1":
                OSCHED = {3: [0, 1], 4: [2, 3], 5: [4, 5], 6: [6, 7],
                          7: [8, 9], 8: [10, 11], 9: [12, 13]}
                OPOST = [14, 15]
            else:
                OSCHED = {3: [0, 1], 4: [2, 3], 5: [4, 5], 6: [6, 7],
                          7: [8, 9]}
                OPOST = list(range(10, 16))

            # Unit 0 runs with a deep ctx lag: its exp stream starts as soon
            # as xq chunk 0 + kT are ready (~8us before xv finishes), and
            # the V projection tiles are hooked in just after xv lands, each
            # one kt slot ahead of the ctx matmul that consumes it.
            LAG0 = min(4, ktk)

            def vhook(kt):
                if LAG0 - 1 <= kt < LAG0 - 1 + ktk:
                    emit_vproj(kt - (LAG0 - 1))

            emit_qproj(0)
            for i, (q0, w, p) in enumerate(units):
                hook = vhook if i == 0 else None
                pcA, pcB = emit_attn(q0, w, p, hook, lag=LAG0 if i == 0 else 2)
                if i in OSCHED:
                    # O-proj m-tiles emitted FIRST so their psum slots
                    # evacuate before the DVE gets busy with the norm chain
                    emit_final(OSCHED[i])
                if i >= 1:
                    pq0, pw, pp_ = units[i - 1]
                    emit_norm(pq0, pw, pp_, *pend[i - 1])
                if i in (1, 3, 5):
                    # project the next q-chunk one unit-pair ahead; the PE
                    # insert hides under the trailing exp queue
                    emit_qproj(i // 2 + 1)
                # recips for this unit go last so the DVE queue serves the
                # previous unit's normalization and evacuations first
                pend[i] = (pcA, pcB, emit_recips(pcA, pcB, w))
            lq0, lw, lp = units[-1]
            emit_norm(lq0, lw, lp, *pend[len(units) - 1])
            emit_final(OPOST)

    nc.compile()
    return nc


def _const_rows():
    cst = np.zeros((3, 128), np.float32)
    cst[0, :] = 1.0
    cst[1, 0:64] = 1.0
    cst[2, 64:128] = 1.0
    return cst


def make_in_maps(query, key, value, mask, Wq, bq, Wk, bk, Wv, bv, Wo, bo):
    """Returns (in_maps, ktk). Key positions with mask=True are dropped."""
    query = np.asarray(query, np.float32)
    key = np.asarray(key, np.float32)
    value = np.asarray(value, np.float32)
    mask = np.asarray(mask)
    Wq = np.asarray(Wq, np.float32)
    Wk = np.asarray(Wk, np.float32)
    Wv = np.asarray(Wv, np.float32)
    Wo = np.asarray(Wo, np.float32)
    bq = np.asarray(bq, np.float32)
    bk = np.asarray(bk, np.float32)
    bv = np.asarray(bv, np.float32)

    keep = [np.flatnonzero(~mask[b, 0]) for b in range(B)]
    ktk = max(1, max((len(k) + 127) // 128 for k in keep))
    SKc = 128 * ktk
    ndt = _np_dt()

    def _prearrange(w):
        # [D, n] -> [128, KT_D, n] matching the SBUF tile layout
        n = w.shape[1]
        return np.ascontiguousarray(
            w.reshape(KT_D, 128, n).transpose(1, 0, 2)
        )

    in_maps = []
    for c in range(NCORES):
        b, g = c // G, c % G
        cs = slice(g * DG, (g + 1) * DG)
        idx = keep[b]
        nk = len(idx)
        xkc = np.zeros((D, SKc), np.float32)
        xvc = np.zeros((D, SKc), np.float32)
        xkc[:, :nk] = key[b].T[:, idx]
        xvc[:, :nk] = value[b].T[:, idx]
        mbias = np.full(SKc, MASK_NEG, np.float32)
        mbias[:nk] = 0.0

        wv_aug = np.zeros((D, VW), np.float32)
        wvb_row = np.zeros((1, VW), np.float32)
        for j in range(HPG):
            src = slice(g * DG + j * DK, g * DG + (j + 1) * DK)
            wv_aug[:, j * 65 : j * 65 + 64] = Wv[:, src]
            wvb_row[0, j * 65 : j * 65 + 64] = bv[src]
            wvb_row[0, j * 65 + 64] = 1.0

        # wo: [DG, D] -> [128, NT, D]
        wo_pre = np.ascontiguousarray(
            Wo[cs, :].reshape(NT, 128, D).transpose(1, 0, 2)
        )

        in_maps.append(
            {
                "xq": np.ascontiguousarray(query[b].T).astype(ndt),
                "xk": xkc.astype(ndt),
                "xv": xvc.astype(ndt),
                "wq": _prearrange(Wq[:, cs]).astype(ndt),
                "wk": _prearrange(Wk[:, cs]).astype(ndt),
                "wv": _prearrange(wv_aug).astype(ndt),
                "wvb": wvb_row.astype(ndt),
                "wo": wo_pre.astype(ndt),
                "bq": np.ascontiguousarray(bq[cs].reshape(NT, 128).T),
                "bk": np.ascontiguousarray(bk[cs].reshape(NT, 128).T),
                "mb": np.ascontiguousarray(mbias.reshape(ktk, 128).T),
                "cstc": _const_rows().astype(ndt),
            }
        )
    return in_maps, ktk


def combine_outputs(results, mask, bo):
    mask = np.asarray(mask)
    bo = np.asarray(bo, np.float32)
    out = np.zeros((B, S, D), np.float32)
    for c in range(NCORES):
        out[c // G] += np.asarray(results[c]["out"], np.float32)
    for b in range(B):
        if mask[b, 0].all():
            # reference: fully-masked rows produce zero context
            out[b] = 0.0
    out += bo[None, None, :]
    return out


_NC_CACHE = {}


def kernel(query, key, value, mask, Wq, bq, Wk, bk, Wv, bv, Wo, bo):
    from concourse.bass_utils import run_bass_kernel_spmd

    in_maps, ktk = make_in_maps(
        query, key, value, mask, Wq, bq, Wk, bk, Wv, bv, Wo, bo
    )
    nc = _NC_CACHE.get((KDT, ktk))
    if nc is None:
        nc = _NC_CACHE[(KDT, ktk)] = build_bass(ktk)
    res = run_bass_kernel_spmd(nc, in_maps, list(range(NCORES))).results
    return combine_outputs(res, mask, bo)
